# revision 9
# baseline (speedup 1.0000x reference)
"""ColBERT MaxSim kernel for 8 Trainium2 NeuronCores (Bass/Tile).

Strategy: data-parallel over the 256-doc batch (32 docs per core).

Host side:
  - compacts each doc's VALID tokens (d_mask is ~50% dense) to the
    front and pads to a per-quad budget with a COPY of the doc's first
    valid token.  Duplicating a valid token leaves the per-(query,doc)
    max unchanged, so this is exactly equivalent to -inf masking.
  - VARIABLE per-quad budgets: each core holds 4 query batches x 8
    docs; each batch's docs are sorted by valid count, and quad g takes
    the rank-g doc of every batch (position d <-> batch d, so the
    kernel's doc->query mapping stays compile-time static).  Budget
    lt[g] = max valid count over quad-slot g across all cores.
  - computes the query side on host in fp32: qp = l2norm(W @ q), then
    packs it into BLOCK lhsT tiles (x16 for fp8 range): qblk[:, A] has
    batch-0 queries in K-half 0 / cols 0-31 and batch-1 queries in
    K-half 1 / cols 32-63, so ONE DoubleRow matmul computes two docs'
    sim tiles at once.  ones2 is the same block layout of ones for the
    per-token sumsq.  W is pre-scaled by 8 for fp8; all scales divide
    out in the host-side final sum.
  - the final sum over the 32 query tokens runs on host (a reduction
    of the DMA'd max table, divided by 16*8).

Per core (32 docs = 8 quads), DMA-bound design (~6.8 MB fp8 in):
  slab DMA triggers all sit on the (otherwise idle) sync queue; the
  slab pool recycles 3 buffers so the DMA for slab g+3 carries a WAR
  wait on quad g's consumption — demand-pacing with 3 slabs of
  lookahead.  In-flight DMAs share bandwidth FAIRLY, so issuing
  everything at once would make slab0 finish LAST; staggering is
  essential.
  per pair of docs (DoubleRow fp8, K=256 per pass):
    pd[:, j] = W8.T @ dT[d]              [128dim, lt] f32 PSUM
    pd8 = fp8(pd)                        (DVE cast -> SBUF)
    sq8 = fp8((pd/8)^2)                  (ACT square w/ scale -> SBUF)
  per quad g (2 doc-pairs; epilogue fires one pair into the next quad),
  everything on PSUM partitions 0-63 (fp8-DR matmuls cannot write at a
  partition offset — s3d3_mm_valid_dst_partition):
    ssq[0:64, h, :] = ones2.T @DR sq8[h]   (per-token sumsq, 2 docs/mm)
    sim[0:64, h, :] = qblk[h].T @DR pd8[h] (2 docs/mm)
    invb = rsqrt(ssq + eps)              (ACT)
    scaled = sim * invb                  (DVE)
    maxcol[:, :, g] = max_tok(scaled)    (DVE reduce_max)
  maxcol [64, 2, 8] is DMA'd out; host sums each 32-query block.
  All sim/ssq matmuls are fp8 DoubleRow: PE work is <1us/quad even at
  1.2 GHz vs ~2.1us/quad of DMA, so no warmup/fillers are needed and
  the result is insensitive to the HAM clock gate.
"""

import numpy as np
import ml_dtypes

import concourse.bass as bass
import concourse.bacc as bacc
import concourse.mybir as mybir
import concourse.tile as tile
from concourse.bass_utils import run_bass_kernel_spmd

N_CORES = 8
H, HC, P = 768, 6, 128   # hidden dim, h-chunks, partitions
DIM = 128                # projection dim
DPC = 32                 # docs per core
QPC = 128                # query vectors per core (4 batches x 32)
PPQ = 8                  # passages per query
NQUAD = DPC // 4
BF16 = mybir.dt.bfloat16
FP8 = mybir.dt.float8e4
F32 = mybir.dt.float32
EPS2 = 1e-12
LT_MIN = 64              # floor on per-quad compacted token budget
W8SCALE = 8.0            # fp8 pre-scale on W; divided out on host
QSCALE = 16.0            # fp8 pre-scale on qp; divided out on host

_LTS = (288,) * NQUAD
_NC_CACHE = {}


def _rsqrt_act(nc, out, in_, bias_ap):
    """out = 1/sqrt(in_ + bias). Emits the Rsqrt activation directly
    (bass's helper refuses it; the 40k-entry reciprocal_sqrt HW table is
    plenty accurate for this kernel's fp8-dominated error budget)."""
    eng = nc.scalar
    ins = [eng.lower_ap(in_), eng.lower_ap(bias_ap),
           mybir.ImmediateValue(dtype=mybir.dt.float32, value=1.0),
           mybir.ImmediateValue(dtype=mybir.dt.float32, value=0.0)]
    return eng.add_instruction(mybir.InstActivation(
        name=nc.get_next_instruction_name(),
        func=mybir.ActivationFunctionType.Rsqrt,
        ins=ins, outs=[eng.lower_ap(out)]))


def _build_nc(lts):
    nc = bacc.Bacc()
    dt_d = [nc.declare_dram_parameter(f"dt{g}", [P, 4, HC, lts[g]], FP8,
                                      isOutput=False) for g in range(NQUAD)]
    qb_d = nc.declare_dram_parameter("qblk", [DIM, 2, 2, 64], FP8,
                                     isOutput=False)
    on_d = nc.declare_dram_parameter("ones2", [DIM, 2, 64], FP8,
                                     isOutput=False)
    wt8_d = nc.declare_dram_parameter("wt8", [P, HC, DIM], FP8,
                                      isOutput=False)
    out_d = nc.declare_dram_parameter("out", [64, 2, NQUAD], F32,
                                      isOutput=True)
    DR = mybir.MatmulPerfMode.DoubleRow
    SQ = mybir.ActivationFunctionType.Square

    with tile.TileContext(nc) as tc:
        with (
            tc.tile_pool(name="const", bufs=1) as const,
            tc.tile_pool(name="slab", bufs=3) as slabp,
            tc.tile_pool(name="work", bufs=2) as work,
            tc.tile_pool(name="psum", bufs=2, space=bass.MemorySpace.PSUM) as psum,
        ):
            # ---- input DMAs (sync queue; slab pool WAR paces g>=3) ----
            wt8_s = const.tile([P, HC, DIM], FP8)
            nc.sync.dma_start(out=wt8_s, in_=wt8_d[:])
            slabs = {}
            for g in range(NQUAD):
                slabs[g] = slabp.tile([P, 4, HC, lts[g]], FP8,
                                      tag="slab", name=f"slab{g}")
            nc.sync.dma_start(out=slabs[0][:, 0:2], in_=dt_d[0][:, 0:2])
            nc.sync.dma_start(out=slabs[0][:, 2:4], in_=dt_d[0][:, 2:4])
            qb_s = const.tile([DIM, 2, 2, 64], FP8)
            nc.sync.dma_start(out=qb_s, in_=qb_d[:])
            on_s = const.tile([DIM, 2, 64], FP8)
            nc.sync.dma_start(out=on_s, in_=on_d[:])
            for g in range(1, NQUAD):
                nc.sync.dma_start(out=slabs[g], in_=dt_d[g][:])

            # ---- constants ----
            eps_t = const.tile([64, 1], F32)       # rsqrt bias (l2norm eps^2)
            nc.vector.memset(eps_t, EPS2)
            maxcol = const.tile([64, 2, NQUAD], F32)

            state = {}

            def emit_epi(g, halves=1, only=None):
                # halves=2 processes doc-pair A then B separately to
                # shorten the serial tail chain of the final quad
                sq8, pd8 = state[g]
                lt = lts[g]
                step = 2 // halves
                for h in range(halves):
                    if only is not None and h != only:
                        continue
                    # halves land in different PSUM banks (free-dim split):
                    # PE writing a bank ACT/DVE read is a fatal collision
                    ssq = psum.tile([64, 2, 512], F32, tag="ssq", bufs=1)
                    sim = psum.tile([64, 2, 512], F32, tag="sim", bufs=1)
                    prs = range(h * step, (h + 1) * step)
                    for pr in prs:
                        nc.tensor.matmul(ssq[:, pr, :lt], on_s, sq8[:, pr],
                                         start=True, stop=True, perf_mode=DR)
                    for pr in prs:
                        nc.tensor.matmul(sim[:, pr, :lt], qb_s[:, pr],
                                         pd8[:, pr],
                                         start=True, stop=True, perf_mode=DR)
                    invb = work.tile([64, 2, lt], F32, tag="invb")
                    scaled = work.tile([64, 2, lt], BF16, tag="scaled")
                    for pr in prs:
                        _rsqrt_act(nc, invb[:, pr], ssq[:, pr, :lt], eps_t)
                        nc.vector.tensor_mul(scaled[:, pr], sim[:, pr, :lt],
                                             invb[:, pr])
                        nc.vector.reduce_max(out=maxcol[:, pr, g:g + 1],
                                             in_=scaled[:, pr],
                                             axis=mybir.AxisListType.X)

            def emit_proj(pp):
                g, lt, slab = pp // 2, lts[pp // 2], slabs[pp // 2]
                pd = psum.tile([DIM, 2, 512], F32, tag="pd")
                for c in range(0, HC, 2):
                    for j in range(2):
                        d = 2 * (pp % 2) + j
                        nc.tensor.matmul(pd[:, j, :lt], wt8_s[:, c:c + 2, :],
                                         slab[:, d, c:c + 2, :],
                                         start=(c == 0), stop=(c == HC - 2),
                                         perf_mode=DR)
                return pd

            def emit_copies(pp, pd):
                g, lt = pp // 2, lts[pp // 2]
                sq8, pd8 = state[g]
                pr = pp % 2
                nc.vector.tensor_copy(pd8[:, pr], pd[:, :, :lt])
                # sq8 = fp8((pd/8)^2): the 1/8 keeps squares ~chi^2(1),
                # centered in e4m3 range
                nc.scalar.activation(sq8[:, pr], pd[:, :, :lt], SQ,
                                     0.0, 1.0 / W8SCALE)

            # ---- doc loop: 16 pairs; quad epilogue fires one pair late ----
            for pp in range(DPC // 2):
                g = pp // 2
                lt = lts[g]
                if pp % 2 == 0:
                    sq8 = work.tile([P, 2, 2, lt], FP8, tag="sq8")
                    pd8 = work.tile([P, 2, 2, lt], FP8, tag="pd8")
                    state[g] = (sq8, pd8)
                if pp == DPC // 2 - 1:
                    # last pair: the previous quad's epilogue goes FIRST so
                    # its work happens while we wait on the final slab's
                    # DMA, then a fine-grained per-half tail.
                    emit_epi(g - 1)
                    pd = emit_proj(pp)
                    emit_copies(pp, pd)
                    emit_epi(g, halves=2, only=0)
                    emit_epi(g, halves=2, only=1)
                else:
                    pd = emit_proj(pp)
                    if pp % 2 == 1 and pp >= 3:
                        emit_epi(g - 1)
                    emit_copies(pp, pd)

            # ---- writeback (host does the 32-query sums) ----
            nc.sync.dma_start(out=out_d[:], in_=maxcol)
    nc.compile()
    return nc


def _get_nc():
    nc = _NC_CACHE.get(_LTS)
    if nc is None:
        nc = _NC_CACHE[_LTS] = _build_nc(_LTS)
    return nc


def _prep_in_maps(q_hidden, d_hidden, W, d_mask):
    global _LTS, _PERM
    f8 = ml_dtypes.float8_e4m3
    cnt = d_mask.sum(1)
    order = np.argsort(~d_mask, axis=1, kind="stable")
    # quad g on every core = the rank-g doc (by valid count) of each of
    # the core's 4 query batches; position within quad = batch index
    perm = np.zeros((N_CORES, NQUAD, 4), dtype=np.int64)
    for c in range(N_CORES):
        for b in range(4):
            docs = np.arange((4 * c + b) * PPQ, (4 * c + b + 1) * PPQ)
            perm[c, :, b] = docs[np.argsort(-cnt[docs], kind="stable")]
    _PERM = perm
    lts = tuple(int(max(LT_MIN, (int(cnt[perm[:, g, :]].max()) + 15)
                        // 16 * 16)) for g in range(NQUAD))
    _LTS = lts
    wt_t = np.ascontiguousarray(W.T.reshape(HC, P, DIM).transpose(1, 0, 2))
    wt8 = (wt_t * W8SCALE).astype(f8)
    # block ones2 [dim, 2, 64]: K-half j feeds output cols 32j..32j+32
    ones2 = np.zeros((DIM, 2, 64), dtype=f8)
    ones2[:, 0, 0:32] = 1.0
    ones2[:, 1, 32:64] = 1.0
    # query side on host: qp = l2norm(W @ q), packed into block lhsT
    qf = q_hidden.reshape(-1, H).astype(np.float32)          # [1024q, H]
    qp = qf @ W.T                                            # [1024q, dim]
    qp /= np.maximum(np.sqrt((qp * qp).sum(-1, keepdims=True)), 1e-12)
    qp *= QSCALE
    in_maps = []
    for c in range(N_CORES):
        m = {"wt8": wt8, "ones2": ones2}
        for g in range(NQUAD):
            lt = lts[g]
            ids = perm[c, g, :]                               # 4 global docs
            idxg = np.where(np.arange(lt)[None, :] >= cnt[ids][:, None],
                            order[ids, :1], order[ids, :lt])
            dcg = np.take_along_axis(d_hidden[ids], idxg[:, :, None], axis=1)
            dtg = dcg.astype(f8).transpose(0, 2, 1)           # [4, 768, lt]
            dtg = dtg.reshape(4, HC, P, lt).transpose(2, 0, 1, 3)
            m[f"dt{g}"] = np.ascontiguousarray(dtg)           # [P, 4, HC, lt]
        qsl = qp[c * QPC:(c + 1) * QPC]                       # [128q, dim]
        qb = np.zeros((DIM, 2, 2, 64), dtype=f8)              # [dim, pr, j, m]
        for pr in range(2):                                   # doc pair in quad
            for j in range(2):                                # K-half = batch
                b = 2 * pr + j
                qb[:, pr, j, 32 * j:32 * j + 32] = \
                    qsl[32 * b:32 * b + 32].T.astype(f8)
        m["qblk"] = qb
        in_maps.append(m)
    return in_maps


def _run(in_maps, trace=False, **kw):
    res = run_bass_kernel_spmd(
        _get_nc(), in_maps, core_ids=list(range(N_CORES)), trace=trace, **kw)
    # per-core output is maxcol [64, 2, NQUAD] (scaled by QSCALE*W8SCALE):
    # row r of half h = query r%32 of batch 2h + r//32; host sums the 32
    # query rows -> score [b, g] for doc _PERM[core, g, b]
    out = np.zeros(N_CORES * DPC, dtype=np.float32)
    for c in range(N_CORES):
        r = res.results[c]["out"].astype(np.float32)          # [64, 2, NQ]
        r = r.reshape(2, 32, 2, NQUAD).sum(axis=1)            # [bw, h, NQ]
        r /= (QSCALE * W8SCALE)
        for g in range(NQUAD):
            for h in range(2):
                for bw in range(2):
                    out[_PERM[c, g, 2 * h + bw]] = r[bw, h, g]
    return out, res


def kernel(q_hidden, d_hidden, W, d_mask, ppq):
    q_hidden = np.asarray(q_hidden, dtype=np.float32)
    d_hidden = np.asarray(d_hidden, dtype=np.float32)
    W = np.asarray(W, dtype=np.float32)
    d_mask = np.asarray(d_mask).astype(bool)
    in_maps = _prep_in_maps(q_hidden, d_hidden, W, d_mask)
    out, _ = _run(in_maps, trace=False)
    return out
